# revision 26
# baseline (speedup 1.0000x reference)
"""LSTM decoder with attention (image captioning) — Trainium2 Bass kernel.

Sharding: data-parallel over batch (64 images -> 8 cores x 8 images).
Collective-free; host does cheap glue (embedding gather, weight
transposes/scaling, h0/c0 init, final bias add).

Key device-side structure (b = 8 local images per core):
  pre:   enc_projT[a,(b,j,q)] = wenc @ IF.T   (fp8 DoubleRow matmuls, 2x)
         IFW[(b,j,q), d4]     = IF @ Wc.T     (fp8 DoubleRow matmuls, 2x)
  loop (t = 0..19, serial):
         hprojT = wdec @ h                       (PE)
         att_pre = encp + hproj  (vector+gpsimd tensor_scalar adds,
                                  packed [a, (b,p)] layout, P=196 packed)
         att = tanh(att_pre)     (scalar, 4 big in-place instrs)
         e_T = att . V           (PE, transposed-e trick)
         softmax via exp + ones-matmul sum + reciprocal
         gates = W_hh@h + embproj[t] + IFW^T w   (PSUM accum;
                 W_hh/embproj parts issued early, overlap the tanh)
         LSTM cell with sigmoid(x) = 0.5(1+tanh(x/2)): H stores 2h and
         C stores 2c so ALL activations are tanh/exp/copy -> they share
         one ACT table (exp_and_others): zero table reloads per step.
         H written directly in bf16 (feeds all matmuls + tail).
  tail:  logits = H_T.T @ (0.5 fc_w).T  (fct prefetched during the loop)

Host pre-scales wdec/W_hh/fc_w by 0.5 and h0/c0 by 2 to absorb the
sigmoid-as-tanh algebra. Recurrence matmuls bf16; accumulation fp32.
"""

import os
import sys
import numpy as np

for _p in ("/opt/trn_rl_repo",):
    if _p not in sys.path and os.path.isdir(_p):
        sys.path.insert(0, _p)

import ml_dtypes  # noqa: E402

import concourse.bass as bass  # noqa: E402
import concourse.tile as tile  # noqa: E402
from concourse import bacc, mybir  # noqa: E402
from concourse.bass import ts  # noqa: E402
from concourse.bass_utils import run_bass_kernel_spmd  # noqa: E402

AF = mybir.ActivationFunctionType
OP = mybir.AluOpType
DR = mybir.MatmulPerfMode.DoubleRow
F32 = mybir.dt.float32
BF16 = mybir.dt.bfloat16
FP8 = mybir.dt.float8e4
BF = ml_dtypes.bfloat16
F8 = ml_dtypes.float8_e4m3

# problem shapes (hardcoded)
VOCAB, ENC, EMB, DEC, ATT = 10000, 2048, 512, 512, 512
B, P, S = 64, 196, 20
NCORES = 8
NB = B // NCORES          # 8 images per core
PPAD = 256                # P padded to 2 k-tiles per image
NJ = PPAD // 128          # 2
NBJ = NB * NJ             # 16 (b,j) row-tiles
NE = ENC // 128           # 16
NKK = NE // 2             # 8 fp8 DoubleRow k-pair tiles
NA = ATT // 128           # 4
ND = DEC // 128           # 4
D4 = 4 * DEC              # 2048
NVC = 20                  # vocab chunks
VC = VOCAB // NVC         # 500
NBH = NB // 2             # 4 images per attention half
HPACK = NBH * P           # 784 packed att columns per half
HPAD = HPACK + 60         # half stride: last image's j=1 e-matmul reads
                          # 60 cols past the half; keep them zero forever
APAD = 2 * HPAD           # 1688 att tile width

_CACHE = {}
TRACE = False  # set by test.py to capture an NTFF profile


def _build_nc():
    if "nc" in _CACHE:
        return _CACHE["nc"]

    nc = bacc.Bacc(
        "TRN2",
        target_bir_lowering=False,
        debug=False,
        enable_asserts=False,
        num_devices=NCORES,
    )

    def din(name, shape, dt=BF16):
        return nc.dram_tensor(name, shape, dt, kind="ExternalInput").ap()

    # fp8 pair layouts (kk pairs e-tiles 2kk / 2kk+1):
    #   ift8[kk][:, c*1024 : c*1024+512]  = ift[2kk][:, c*512:+512]
    #   ift8[kk][:, c*1024+512 : +1024]   = ift[2kk+1][:, c*512:+512]
    ift8_d = din("ift8", [NKK, 128, 2 * NB * PPAD], FP8)
    wct8_d = din("wct8", [NKK, 128, 2 * D4], FP8)
    wenct8_d = din("wenct8", [NKK, 128, 2 * ATT], FP8)
    whht_d = din("whht", [ND, 128, D4])             # (0.5 W_hh).T
    wdect_d = din("wdect", [ND, 128, ATT])          # (0.5 wdec).T
    vt_d = din("vt", [NA, 128, 1])                  # V_w.T
    ept_d = din("ept", [128, S * 4 * 4 * NB])       # embproj [r,(t,gate,r4,b)]
    i128_d = din("i128", [128, 128])                # identity bf16
    fct_d = din("fct", [ND, 128, VOCAB])            # (0.5 fc_w).T
    h0t_d = din("h0t", [ND, 128, NB])               # 2*h0, bf16
    c0t_d = din("c0t", [128, 4 * NB], F32)          # 2*c0, cols (r,b)
    encb_d = din("encb", [NA, 128, 1], F32)         # wenc_b + wdec_b
    out_d = nc.dram_tensor("out", [S * NB, VOCAB], F32, kind="ExternalOutput").ap()

    with tile.TileContext(nc) as tc:
        from contextlib import ExitStack

        with ExitStack() as glob_ctx:
            gp = glob_ctx.enter_context(tc.tile_pool(name="glob", bufs=1))
            # persistent state / loop constants
            # IFW split by gate precision: i/f/o gates (saturating sigmoids)
            # tolerate fp8 ctx weights; the tanh'd g gate needs bf16.
            # ifw8 cols [0:1024] = d4 [0:1024] (i,f), [1024:1536] = d4
            # [1536:2048] (o); ifwg = d4 [1024:1536] (g).
            ifw8 = [gp.tile([128, 1536], FP8, name=f"ifw8_{i}",
                            tag=f"ifw8_{i}") for i in range(NBJ)]
            ifwg = [gp.tile([128, 512], BF16, name=f"ifwg{i}",
                            tag=f"ifwg{i}") for i in range(NBJ)]
            encp = [gp.tile([128, NB * PPAD], BF16, name=f"encp{i}",
                            tag=f"encp{i}") for i in range(NA)]
            HT = [gp.tile([128, (S + 1) * NB], BF16, name=f"HT{i}",
                          tag=f"HT{i}") for i in range(ND)]
            cC = gp.tile([128, 4 * NB], F32, name="cC")
            tI = gp.tile([128, 4 * NB], F32, name="tI")
            tF = gp.tile([128, 4 * NB], F32, name="tF")
            tG = gp.tile([128, 4 * NB], F32, name="tG")
            tO = gp.tile([128, 4 * NB], F32, name="tO")
            tA = gp.tile([128, 4 * NB], F32, name="tA")
            tB = gp.tile([128, 4 * NB], F32, name="tB")
            hp_s = gp.tile([128, NA * NB], BF16, name="hp_s")
            i128 = gp.tile([128, 128], BF16, name="i128")
            vt = [gp.tile([128, 1], BF16, name=f"vt{i}", tag=f"vt{i}")
                  for i in range(NA)]
            encb = [gp.tile([128, 1], F32, name=f"encb{i}", tag=f"encb{i}")
                    for i in range(NA)]
            ones_col = gp.tile([128, 1], F32, name="ones_col")
            ones_row = gp.tile([1, 128], F32, name="ones_row")
            zeros16 = gp.tile([128, NJ * NB], BF16, name="zeros16")
            et_exp = gp.tile([128, NJ * NB], F32, name="et_exp")
            rsum_sb = gp.tile([1, NB], F32, name="rsum_sb")
            wt_sb = [gp.tile([128, NB], BF16, name=f"wt{j}", tag=f"wt{j}")
                     for j in range(NJ)]
            wt_f8 = [gp.tile([128, NB], FP8, name=f"wt8{j}", tag=f"wt8{j}")
                     for j in range(NJ)]
            whht = [gp.tile([128, D4], BF16, name=f"whht{k}", tag=f"whht{k}")
                    for k in range(ND)]
            wdect = [gp.tile([128, ATT], BF16, name=f"wdect{k}",
                             tag=f"wdect{k}") for k in range(ND)]
            ept = gp.tile([128, S * 4 * 4 * NB], BF16, name="ept")

            nc.sync.dma_start(out=i128, in_=i128_d)
            for i in range(NA):
                nc.sync.dma_start(out=vt[i], in_=vt_d[i])
                nc.sync.dma_start(out=encb[i], in_=encb_d[i])
            nc.vector.memset(ones_col, 1.0)
            nc.vector.memset(ones_row, 1.0)
            nc.vector.memset(zeros16, 0.0)

            # ---------------- pre-loop: enc_proj and IFW (fp8 DR) --------
            with tc.tile_pool(name="pre", bufs=1) as pre, \
                 tc.tile_pool(name="pspre", bufs=4, space="PSUM") as pspre:
                wenct8 = [pre.tile([128, 2 * ATT], FP8, name=f"we8{k}",
                                   tag=f"we8{k}") for k in range(NKK)]
                ift8 = [pre.tile([128, 2 * NB * PPAD], FP8, name=f"if8{k}",
                                 tag=f"if8{k}") for k in range(NKK)]
                wct8 = [pre.tile([128, 2 * D4], FP8, name=f"wc8{k}",
                                 tag=f"wc8{k}") for k in range(NKK)]
                for k in range(NKK):
                    nc.sync.dma_start(out=wenct8[k], in_=wenct8_d[k])
                for k in range(NKK):
                    nc.sync.dma_start(out=ift8[k], in_=ift8_d[k])
                for k in range(NKK):
                    nc.sync.dma_start(out=wct8[k], in_=wct8_d[k])
                # recurrence weights (arrive during pre-loop compute)
                for k in range(ND):
                    nc.sync.dma_start(out=wdect[k], in_=wdect_d[k])
                    nc.sync.dma_start(out=whht[k], in_=whht_d[k])
                for k in range(ND):
                    nc.sync.dma_start(out=HT[k][:, 0:NB], in_=h0t_d[k])
                nc.sync.dma_start(out=cC, in_=c0t_d)
                nc.sync.dma_start(out=ept, in_=ept_d)

                # pair views: [p, (c two n)] -> index c, pair-split two
                ift8v = [t.rearrange("p (c two n) -> p c two n",
                                     c=4, two=2, n=512) for t in ift8]
                # for IFW lhsT: [p, (c two k f)] -> p c k two f
                ift8w = [t.rearrange("p (c two k f) -> p c k two f",
                                     c=4, two=2, k=4, f=128) for t in ift8]
                wct8v = [t.rearrange("p (c two n) -> p c two n",
                                     c=4, two=2, n=512) for t in wct8]
                wenct8v = [t.rearrange("p (i two f) -> p i two f",
                                       i=4, two=2, f=128) for t in wenct8]

                # enc_projT [a, (b,j,q)] += wenc_b + wdec_b
                for i in range(NA):
                    for c in range(NB * PPAD // 512):
                        ps = pspre.tile([128, 512], F32, name="eps", tag="mm")
                        for kk in range(NKK):
                            nc.tensor.matmul(
                                ps, wenct8v[kk][:, i], ift8v[kk][:, c],
                                start=(kk == 0), stop=(kk == NKK - 1),
                                perf_mode=DR)
                        nc.vector.tensor_scalar_add(
                            encp[i][:, ts(c, 512)], ps, encb[i])

                # IFW[(b,j,q), d4] = IF @ Wc.T; chunks c=0,1 (i,f) and 3 (o)
                # stored fp8, chunk 2 (g gate) bf16
                for bj in range(NBJ):
                    for c in range(4):
                        ps = pspre.tile([128, 512], F32, name="wps", tag="mm")
                        for kk in range(NKK):
                            nc.tensor.matmul(
                                ps, ift8w[kk][:, bj // 4, bj % 4],
                                wct8v[kk][:, c],
                                start=(kk == 0), stop=(kk == NKK - 1),
                                perf_mode=DR)
                        if c == 2:
                            dst = ifwg[bj]
                        elif c == 3:
                            dst = ifw8[bj][:, 1024:1536]
                        else:
                            dst = ifw8[bj][:, ts(c, 512)]
                        if (bj + c) % 2 == 0:
                            nc.scalar.copy(out=dst, in_=ps)
                        else:
                            nc.vector.tensor_copy(out=dst, in_=ps)

            # ---------------- recurrence + tail ----------------
            with tc.tile_pool(name="rec", bufs=1) as rec, \
                 tc.tile_pool(name="psl", bufs=1, space="PSUM") as psl:
                att = [rec.tile([128, APAD], BF16, name=f"att{i}",
                                tag=f"att{i}") for i in range(NA)]
                fct = [rec.tile([128, VOCAB], BF16, name=f"fct{k}",
                                tag=f"fct{k}") for k in range(ND)]
                # zero the per-half pad columns once (the last image of each
                # half has its j=1 e-matmul read 60 cols past the half)
                for i in range(NA):
                    for h in range(2):
                        nc.vector.memset(
                            att[i][:, h * HPAD + HPACK:(h + 1) * HPAD], 0.0)
                for k in range(ND):
                    nc.sync.dma_start(out=fct[k], in_=fct_d[k])

                # strided views for the one-instr-per-i bias add
                encv = [encp[i].rearrange("p (b q) -> p b q", b=NB, q=PPAD)
                        [:, :, 0:P] for i in range(NA)]
                attv = [[att[i][:, h * HPAD:h * HPAD + HPACK]
                         .rearrange("p (b q) -> p b q", b=NBH, q=P)
                         for h in range(2)] for i in range(NA)]

                for t in range(S):
                    hof = t * NB
                    hofn = hof + NB
                    # one aux PSUM bank: hp [0:32], etp [32:48], rs [48:56],
                    # sum [56:64] (start=True poisons a whole bank, so all
                    # starts here precede any re-accumulation of live bytes)
                    aux = psl.tile([128, 64], F32, name="aux", tag="aux",
                                   bufs=2)
                    hp_ps = aux[:, 0:NA * NB]
                    etp = aux[:, 32:32 + NJ * NB]
                    rs_ps = aux[:, 48:48 + NB]
                    sum_ps = aux[0:1, 56:56 + NB]
                    # hproj (i-outer so tsa/tanh of i=0 start early; also
                    # keeps each i-region's start->accum sequence contiguous)
                    for i in range(NA):
                        for k in range(ND):
                            nc.tensor.matmul(
                                hp_ps[:, ts(i, NB)], wdect[k][:, ts(i, 128)],
                                HT[k][:, hof:hofn], start=(k == 0),
                                stop=(k == ND - 1), skip_group_check=True)
                        nc.vector.tensor_copy(out=hp_s[:, ts(i, NB)],
                                              in_=hp_ps[:, ts(i, NB)])
                    # gates: per-r PSUM tiles (tile==bank; keeps ctx writes
                    # and cell reads on different tiles so they pipeline).
                    # One ept matmul per r with start=True initializes the
                    # bank; then W_hh (k-outer: pipelines with the previous
                    # step's cell) and ctx accumulate.
                    g_r = [psl.tile([128, 4 * NB], F32, name=f"g{r}",
                                    tag=f"g{r}", bufs=1) for r in range(ND)]
                    for r in range(ND):
                        ec = (t * 4 + r) * 4 * NB
                        nc.tensor.matmul(g_r[r], i128,
                                         ept[:, ec:ec + 4 * NB],
                                         start=True, stop=False,
                                         skip_group_check=True)
                    for k in range(ND):
                        for g in range(4):
                            for r in range(ND):
                                nc.tensor.matmul(
                                    g_r[r][:, ts(g, NB)],
                                    whht[k][:, ts(g * 4 + r, 128)],
                                    HT[k][:, hof:hofn],
                                    start=False, stop=False,
                                    skip_group_check=True)
                    # etp bank-init: one start=True matmul zeroes all 16 e
                    # columns so the e-matmuls can accumulate i-outer (each
                    # i issued right after its tanh) in any order.
                    nc.tensor.matmul(etp, i128, zeros16, start=True,
                                     stop=False, skip_group_check=True)

                    # Attention + softmax + ctx in TWO image-halves: while
                    # half-0's ctx matmuls run on the PE, half-1's add/tanh
                    # spine runs on vector/scalar — hides the serial spine.
                    def e_mm(h, i):
                        # e_T[q, (j,b)] = V . att — contribution of one i
                        for bh in range(NBH):
                            cb = h * HPAD + bh * P
                            for j in range(NJ):
                                col = h * 8 + j * NBH + bh
                                nc.tensor.matmul(
                                    etp[:, col:col + 1],
                                    att[i][:, cb + j * 128:
                                           cb + j * 128 + 128],
                                    vt[i], start=False,
                                    stop=(i == NA - 1),
                                    skip_group_check=True)

                    def spine(h, with_e):
                        b0 = h * NBH
                        for i in range(NA):
                            hv = hp_s[:, i * NB + b0:i * NB + b0 + NBH] \
                                .unsqueeze(2).broadcast_to([128, NBH, P])
                            nc.vector.tensor_tensor(
                                out=attv[i][h],
                                in0=encv[i][:, b0:b0 + NBH],
                                in1=hv, op=OP.add)
                            # att = tanh(att_pre), in place
                            nc.scalar.activation(
                                att[i][:, h * HPAD:h * HPAD + HPACK],
                                att[i][:, h * HPAD:h * HPAD + HPACK],
                                AF.Tanh)
                            if with_e:
                                e_mm(h, i)

                    def sm_sum(h):
                        # softmax over P (no max-subtraction: |e| <= ~11)
                        nc.scalar.activation(et_exp[:, ts(h, 8)],
                                             etp[:, ts(h, 8)], AF.Exp)
                        nc.tensor.matmul(sum_ps[:, ts(h, NBH)],
                                         ones_col[0:128],
                                         et_exp[:, h * 8:h * 8 + NBH],
                                         start=True, stop=False,
                                         skip_group_check=True)
                        nc.tensor.matmul(sum_ps[:, ts(h, NBH)],
                                         ones_col[0:P - 128],
                                         et_exp[0:P - 128,
                                                h * 8 + NBH:h * 8 + 2 * NBH],
                                         start=False, stop=True,
                                         skip_group_check=True)
                        nc.vector.reciprocal(rsum_sb[:, ts(h, NBH)],
                                             sum_ps[:, ts(h, NBH)])

                    def sm_wt(h):
                        b0 = h * NBH
                        nc.tensor.matmul(rs_ps[:, ts(h, NBH)], ones_row,
                                         rsum_sb[:, ts(h, NBH)],
                                         start=True, stop=True,
                                         skip_group_check=True)
                        for j in range(NJ):
                            nc.vector.tensor_mul(
                                wt_sb[j][:, b0:b0 + NBH],
                                et_exp[:, h * 8 + j * NBH:
                                       h * 8 + j * NBH + NBH],
                                rs_ps[:, ts(h, NBH)])
                            nc.vector.tensor_mul(
                                wt_f8[j][:, b0:b0 + NBH],
                                et_exp[:, h * 8 + j * NBH:
                                       h * 8 + j * NBH + NBH],
                                rs_ps[:, ts(h, NBH)])

                    def ctx(h, rr):
                        b0 = h * NBH
                        for r in rr:
                            for g in range(4):
                                if g == 2:
                                    lhs = [ifwg[b * NJ + j][:, ts(r, 128)]
                                           for b in range(b0, b0 + NBH)
                                           for j in range(NJ)]
                                    wts = wt_sb
                                else:
                                    m8 = g * 4 + r if g < 2 else 8 + r
                                    lhs = [ifw8[b * NJ + j][:, ts(m8, 128)]
                                           for b in range(b0, b0 + NBH)
                                           for j in range(NJ)]
                                    wts = wt_f8
                                for bi, b in enumerate(range(b0, b0 + NBH)):
                                    for j in range(NJ):
                                        nc.tensor.matmul(
                                            g_r[r][:, g * NB + b:
                                                   g * NB + b + 1],
                                            lhs[bi * NJ + j],
                                            wts[j][:, b:b + 1],
                                            start=False, stop=(j == NJ - 1),
                                            skip_group_check=True)

                    spine(0, True)     # tt/tanh/e for images 0-3
                    sm_sum(0)          # exp + sum + recip
                    sm_wt(0)           # rs broadcast + weight muls
                    spine(1, False)    # h1 tt/tanh overlap ctx-h0 (vec/scal)
                    ctx(0, (0, 1))
                    for i in range(NA):
                        e_mm(1, i)     # tanh-h1 done by now: no PE stall
                    sm_sum(1)
                    ctx(0, (2, 3))
                    sm_wt(1)
                    for r in range(ND):
                        ctx(1, (r,))
                        # cell for this r (all-tanh: sigmoid(x)=.5(1+tanh(x/2));
                        # H holds 2h, C holds 2c)
                        cs = slice(r * NB, r * NB + NB)
                        nc.scalar.activation(tI[:, cs], g_r[r][:, ts(0, NB)],
                                             AF.Tanh, scale=0.5)
                        nc.scalar.activation(tF[:, cs], g_r[r][:, ts(1, NB)],
                                             AF.Tanh, scale=0.5)
                        nc.scalar.activation(tG[:, cs], g_r[r][:, ts(2, NB)],
                                             AF.Tanh)
                        nc.scalar.activation(tO[:, cs], g_r[r][:, ts(3, NB)],
                                             AF.Tanh, scale=0.5)
                        nc.vector.scalar_tensor_tensor(
                            tA[:, cs], tF[:, cs], 1.0, cC[:, cs],
                            OP.add, OP.mult)
                        nc.vector.scalar_tensor_tensor(
                            tB[:, cs], tI[:, cs], 1.0, tG[:, cs],
                            OP.add, OP.mult)
                        nc.vector.scalar_tensor_tensor(
                            cC[:, cs], tA[:, cs], 0.5, tB[:, cs],
                            OP.mult, OP.add)
                        nc.scalar.activation(tA[:, cs], cC[:, cs],
                                             AF.Tanh, scale=0.5)
                        nc.vector.scalar_tensor_tensor(
                            HT[r][:, hofn:hofn + NB], tO[:, cs], 1.0,
                            tA[:, cs], OP.add, OP.mult)

                # ---------------- tail: logits ----------------
                with tc.tile_pool(name="pst", bufs=2, space="PSUM") as pst:
                    for m0, msz in ((0, 128), (128, S * NB - 128)):
                        for c in range(NVC):
                            ps = pst.tile([128, VC], F32, name="lps", tag="l")
                            for k in range(ND):
                                nc.tensor.matmul(
                                    ps[:msz],
                                    HT[k][:, NB + m0:NB + m0 + msz],
                                    fct[k][:, ts(c, VC)],
                                    start=(k == 0), stop=(k == ND - 1))
                            lg = rec.tile([128, VC], F32, name="lg", tag="lg",
                                          bufs=4)
                            if c % 2 == 0:
                                nc.scalar.copy(out=lg[:msz], in_=ps[:msz])
                            else:
                                nc.vector.tensor_copy(out=lg[:msz],
                                                      in_=ps[:msz])
                            nc.sync.dma_start(
                                out=out_d[m0:m0 + msz, ts(c, VC)],
                                in_=lg[:msz])

    nc.compile()
    _CACHE["nc"] = nc
    return nc


def _prep_core_inputs(image_feat, embproj, h0, c0, wct8, wenct8, whht, wdect,
                      vt, i128, fct, encb, core):
    bs = slice(core * NB, (core + 1) * NB)
    ifp = np.zeros((NB, PPAD, ENC), np.float32)
    ifp[:, :P, :] = image_feat[bs]
    # IF.T as e-tiles [NE, 128, (b,j,q)], then fp8 pair layout
    iftT = np.ascontiguousarray(
        ifp.reshape(NB * PPAD, ENC).T).reshape(NE, 128, NB * PPAD)
    ift8 = np.ascontiguousarray(
        iftT.reshape(NKK, 2, 128, 4, 512).transpose(0, 2, 3, 1, 4)
        .reshape(NKK, 128, 2 * NB * PPAD)).astype(F8)
    ep = embproj[bs]                                   # [8, 20, 2048]
    ept = np.ascontiguousarray(
        ep.transpose(2, 1, 0)                          # [2048, 20, 8]
        .reshape(4, 4, 128, S, NB)                     # [gate, r, row, t, b]
        .transpose(2, 3, 1, 0, 4)                      # [row, t, r, gate, b]
        .reshape(128, S * 16 * NB)).astype(BF)
    h0t = np.ascontiguousarray(
        (2.0 * h0[bs]).T).reshape(ND, 128, NB).astype(BF)
    c0t = np.ascontiguousarray(
        (2.0 * c0[bs]).T.reshape(ND, 128, NB).transpose(1, 0, 2)
        .reshape(128, ND * NB)).astype(np.float32)
    return dict(ift8=ift8, wct8=wct8, wenct8=wenct8, whht=whht, wdect=wdect,
                vt=vt, ept=ept, i128=i128, fct=fct, h0t=h0t, c0t=c0t,
                encb=encb)


def kernel(image_feat, captions_ids, wenc_w, wenc_b, wdec_w, wdec_b,
           V_w, V_b, embed_w, h0_w, h0_b, c0_w, c0_b,
           W_ih, b_ih, W_hh, b_hh, fc_w, fc_b):
    image_feat = np.asarray(image_feat, np.float32)
    ids = np.asarray(captions_ids).astype(np.int64)

    # host-side glue (cheap, not on the device critical path)
    emb_seq = np.asarray(embed_w, np.float32)[ids]            # [B, S, EMB]
    We = np.asarray(W_ih, np.float32)[:, ENC:]                # [D4, EMB]
    Wc = np.asarray(W_ih, np.float32)[:, :ENC]                # [D4, ENC]
    embproj = emb_seq @ We.T + (np.asarray(b_ih) + np.asarray(b_hh))
    avg = image_feat.mean(axis=1)
    h0 = np.maximum(avg @ np.asarray(h0_w, np.float32).T + h0_b, 0.0)
    c0 = np.maximum(avg @ np.asarray(c0_w, np.float32).T + c0_b, 0.0)

    # fp8 pair layouts for the DoubleRow pre-loop matmuls
    wctT = np.ascontiguousarray(Wc.T).reshape(NE, 128, D4)
    wct8 = np.ascontiguousarray(
        wctT.reshape(NKK, 2, 128, 4, 512).transpose(0, 2, 3, 1, 4)
        .reshape(NKK, 128, 2 * D4)).astype(F8)
    wencT = np.ascontiguousarray(
        np.asarray(wenc_w, np.float32).T).reshape(NE, 128, ATT)
    wenct8 = np.ascontiguousarray(
        wencT.reshape(NKK, 2, 128, 4, 128).transpose(0, 2, 3, 1, 4)
        .reshape(NKK, 128, 2 * ATT)).astype(F8)
    whht = np.ascontiguousarray(
        0.5 * np.asarray(W_hh, np.float32).T).astype(BF).reshape(ND, 128, D4)
    wdect = np.ascontiguousarray(
        0.5 * np.asarray(wdec_w, np.float32).T).astype(BF).reshape(
            ND, 128, ATT)
    vtt = np.ascontiguousarray(
        np.asarray(V_w, np.float32)[0]).astype(BF).reshape(NA, 128, 1)
    i128 = np.eye(128, dtype=BF)
    fct = np.ascontiguousarray(
        0.5 * np.asarray(fc_w, np.float32).T).astype(BF).reshape(
            ND, 128, VOCAB)
    encb = (np.asarray(wenc_b, np.float32)
            + np.asarray(wdec_b, np.float32)).reshape(NA, 128, 1)

    nc = _build_nc()
    in_maps = [
        _prep_core_inputs(image_feat, embproj, h0, c0, wct8, wenct8, whht,
                          wdect, vtt, i128, fct, encb, c)
        for c in range(NCORES)
    ]
    res = run_bass_kernel_spmd(nc, in_maps, core_ids=list(range(NCORES)),
                               trace=TRACE)
    if TRACE:
        _CACHE["last_results"] = res

    preds = np.empty((B, S, VOCAB), np.float32)
    for c in range(NCORES):
        lg = res.results[c]["out"].reshape(S, NB, VOCAB)
        preds[c * NB:(c + 1) * NB] = lg.transpose(1, 0, 2)
    preds += np.asarray(fc_b, np.float32)
    return preds


if __name__ == "__main__":
    sys.path.insert(0, os.path.dirname(os.path.abspath(__file__)))
    import reference

    inputs = reference.setup_inputs()
    inputs = {k: np.asarray(v) for k, v in inputs.items()}
    expected = np.asarray(reference.reference(**inputs))
    actual = kernel(**inputs)
    err = np.abs(actual - expected)
    rel = np.linalg.norm(actual - expected) / np.linalg.norm(expected)
    print("max abs err:", err.max(), "rel:", rel)


# revision 27
# speedup vs baseline: 1.0183x; 1.0183x over previous
"""LSTM decoder with attention (image captioning) — Trainium2 Bass kernel.

Sharding: data-parallel over batch (64 images -> 8 cores x 8 images).
Collective-free; host does cheap glue (embedding gather, weight
transposes/scaling, h0/c0 init, final bias add).

Key device-side structure (b = 8 local images per core):
  pre:   enc_projT[a,(b,j,q)] = wenc @ IF.T   (fp8 DoubleRow matmuls, 2x)
         IFW[(b,j,q), d4]     = IF @ Wc.T     (fp8 DoubleRow matmuls, 2x)
  loop (t = 0..19, serial):
         hprojT = wdec @ h                       (PE)
         att_pre = encp + hproj  (vector+gpsimd tensor_scalar adds,
                                  packed [a, (b,p)] layout, P=196 packed)
         att = tanh(att_pre)     (scalar, 4 big in-place instrs)
         e_T = att . V           (PE, transposed-e trick)
         softmax via exp + ones-matmul sum + reciprocal
         gates = W_hh@h + embproj[t] + IFW^T w   (PSUM accum;
                 W_hh/embproj parts issued early, overlap the tanh)
         LSTM cell with sigmoid(x) = 0.5(1+tanh(x/2)): H stores 2h and
         C stores 2c so ALL activations are tanh/exp/copy -> they share
         one ACT table (exp_and_others): zero table reloads per step.
         H written directly in bf16 (feeds all matmuls + tail).
  tail:  logits = H_T.T @ (0.5 fc_w).T  (fct prefetched during the loop)

Host pre-scales wdec/W_hh/fc_w by 0.5 and h0/c0 by 2 to absorb the
sigmoid-as-tanh algebra. Recurrence matmuls bf16; accumulation fp32.
"""

import os
import sys
import numpy as np

for _p in ("/opt/trn_rl_repo",):
    if _p not in sys.path and os.path.isdir(_p):
        sys.path.insert(0, _p)

import ml_dtypes  # noqa: E402

import concourse.bass as bass  # noqa: E402
import concourse.tile as tile  # noqa: E402
from concourse import bacc, mybir  # noqa: E402
from concourse.bass import ts  # noqa: E402
from concourse.bass_utils import run_bass_kernel_spmd  # noqa: E402

AF = mybir.ActivationFunctionType
OP = mybir.AluOpType
DR = mybir.MatmulPerfMode.DoubleRow
F32 = mybir.dt.float32
BF16 = mybir.dt.bfloat16
FP8 = mybir.dt.float8e4
BF = ml_dtypes.bfloat16
F8 = ml_dtypes.float8_e4m3

# problem shapes (hardcoded)
VOCAB, ENC, EMB, DEC, ATT = 10000, 2048, 512, 512, 512
B, P, S = 64, 196, 20
NCORES = 8
NB = B // NCORES          # 8 images per core
PPAD = 256                # P padded to 2 k-tiles per image
NJ = PPAD // 128          # 2
NBJ = NB * NJ             # 16 (b,j) row-tiles
NE = ENC // 128           # 16
NKK = NE // 2             # 8 fp8 DoubleRow k-pair tiles
NA = ATT // 128           # 4
ND = DEC // 128           # 4
D4 = 4 * DEC              # 2048
NVC = 20                  # vocab chunks
VC = VOCAB // NVC         # 500
NBH = NB // 2             # 4 images per attention half
HPACK = NBH * P           # 784 packed att columns per half
HPAD = HPACK + 60         # half stride: last image's j=1 e-matmul reads
                          # 60 cols past the half; keep them zero forever
APAD = 2 * HPAD           # 1688 att tile width

_CACHE = {}
TRACE = False  # set by test.py to capture an NTFF profile


def _build_nc():
    if "nc" in _CACHE:
        return _CACHE["nc"]

    nc = bacc.Bacc(
        "TRN2",
        target_bir_lowering=False,
        debug=False,
        enable_asserts=False,
        num_devices=NCORES,
    )

    def din(name, shape, dt=BF16):
        return nc.dram_tensor(name, shape, dt, kind="ExternalInput").ap()

    # fp8 pair layouts (kk pairs e-tiles 2kk / 2kk+1):
    #   ift8[kk][:, c*1024 : c*1024+512]  = ift[2kk][:, c*512:+512]
    #   ift8[kk][:, c*1024+512 : +1024]   = ift[2kk+1][:, c*512:+512]
    ift8_d = din("ift8", [NKK, 128, 2 * NB * PPAD], FP8)
    wct8_d = din("wct8", [NKK, 128, 2 * D4], FP8)
    wenct8_d = din("wenct8", [NKK, 128, 2 * ATT], FP8)
    whht_d = din("whht", [ND, 128, D4])             # (0.5 W_hh).T
    wdect_d = din("wdect", [ND, 128, ATT])          # (0.5 wdec).T
    vt_d = din("vt", [NA, 128, 1])                  # V_w.T
    ept_d = din("ept", [128, S * 4 * 4 * NB])       # embproj [r,(t,gate,r4,b)]
    i128_d = din("i128", [128, 128])                # identity bf16
    fct_d = din("fct", [ND, 128, VOCAB])            # (0.5 fc_w).T
    h0t_d = din("h0t", [ND, 128, NB])               # 2*h0, bf16
    c0t_d = din("c0t", [128, 4 * NB], F32)          # 2*c0, cols (r,b)
    encb_d = din("encb", [NA, 128, 1], F32)         # wenc_b + wdec_b
    out_d = nc.dram_tensor("out", [S * NB, VOCAB], F32, kind="ExternalOutput").ap()

    with tile.TileContext(nc) as tc:
        from contextlib import ExitStack

        with ExitStack() as glob_ctx:
            gp = glob_ctx.enter_context(tc.tile_pool(name="glob", bufs=1))
            # persistent state / loop constants
            ifw = [gp.tile([128, D4], BF16, name=f"ifw{i}", tag=f"ifw{i}")
                   for i in range(NBJ)]
            encp = [gp.tile([128, NB * PPAD], BF16, name=f"encp{i}",
                            tag=f"encp{i}") for i in range(NA)]
            HT = [gp.tile([128, (S + 1) * NB], BF16, name=f"HT{i}",
                          tag=f"HT{i}") for i in range(ND)]
            cC = gp.tile([128, 4 * NB], F32, name="cC")
            tI = gp.tile([128, 4 * NB], F32, name="tI")
            tF = gp.tile([128, 4 * NB], F32, name="tF")
            tG = gp.tile([128, 4 * NB], F32, name="tG")
            tO = gp.tile([128, 4 * NB], F32, name="tO")
            tA = gp.tile([128, 4 * NB], F32, name="tA")
            tB = gp.tile([128, 4 * NB], F32, name="tB")
            hp_s = gp.tile([128, NA * NB], BF16, name="hp_s")
            i128 = gp.tile([128, 128], BF16, name="i128")
            vt = [gp.tile([128, 1], BF16, name=f"vt{i}", tag=f"vt{i}")
                  for i in range(NA)]
            encb = [gp.tile([128, 1], F32, name=f"encb{i}", tag=f"encb{i}")
                    for i in range(NA)]
            ones_col = gp.tile([128, 1], F32, name="ones_col")
            ones_row = gp.tile([1, 128], F32, name="ones_row")
            zeros16 = gp.tile([128, NJ * NB], BF16, name="zeros16")
            et_exp = gp.tile([128, NJ * NB], F32, name="et_exp")
            rsum_sb = gp.tile([1, NB], F32, name="rsum_sb")
            wt_sb = [gp.tile([128, NB], BF16, name=f"wt{j}", tag=f"wt{j}")
                     for j in range(NJ)]
            whht = [gp.tile([128, D4], BF16, name=f"whht{k}", tag=f"whht{k}")
                    for k in range(ND)]
            wdect = [gp.tile([128, ATT], BF16, name=f"wdect{k}",
                             tag=f"wdect{k}") for k in range(ND)]
            ept = gp.tile([128, S * 4 * 4 * NB], BF16, name="ept")

            nc.sync.dma_start(out=i128, in_=i128_d)
            for i in range(NA):
                nc.sync.dma_start(out=vt[i], in_=vt_d[i])
                nc.sync.dma_start(out=encb[i], in_=encb_d[i])
            nc.vector.memset(ones_col, 1.0)
            nc.vector.memset(ones_row, 1.0)
            nc.vector.memset(zeros16, 0.0)

            # ---------------- pre-loop: enc_proj and IFW (fp8 DR) --------
            with tc.tile_pool(name="pre", bufs=1) as pre, \
                 tc.tile_pool(name="pspre", bufs=4, space="PSUM") as pspre:
                wenct8 = [pre.tile([128, 2 * ATT], FP8, name=f"we8{k}",
                                   tag=f"we8{k}") for k in range(NKK)]
                ift8 = [pre.tile([128, 2 * NB * PPAD], FP8, name=f"if8{k}",
                                 tag=f"if8{k}") for k in range(NKK)]
                wct8 = [pre.tile([128, 2 * D4], FP8, name=f"wc8{k}",
                                 tag=f"wc8{k}") for k in range(NKK)]
                for k in range(NKK):
                    nc.sync.dma_start(out=wenct8[k], in_=wenct8_d[k])
                # c-block-interleaved loads: the (i, c=0) matmul group only
                # needs the first 1MB of ift8, so the PE starts ~15us sooner
                for c in range(4):
                    for k in range(NKK):
                        nc.sync.dma_start(
                            out=ift8[k][:, c * 1024:(c + 1) * 1024],
                            in_=ift8_d[k][:, c * 1024:(c + 1) * 1024])
                for c in range(4):
                    for k in range(NKK):
                        nc.sync.dma_start(
                            out=wct8[k][:, c * 1024:(c + 1) * 1024],
                            in_=wct8_d[k][:, c * 1024:(c + 1) * 1024])
                # recurrence weights (arrive during pre-loop compute)
                for k in range(ND):
                    nc.sync.dma_start(out=wdect[k], in_=wdect_d[k])
                    nc.sync.dma_start(out=whht[k], in_=whht_d[k])
                for k in range(ND):
                    nc.sync.dma_start(out=HT[k][:, 0:NB], in_=h0t_d[k])
                nc.sync.dma_start(out=cC, in_=c0t_d)
                nc.sync.dma_start(out=ept, in_=ept_d)

                # pair views: [p, (c two n)] -> index c, pair-split two
                ift8v = [t.rearrange("p (c two n) -> p c two n",
                                     c=4, two=2, n=512) for t in ift8]
                # for IFW lhsT: [p, (c two k f)] -> p c k two f
                ift8w = [t.rearrange("p (c two k f) -> p c k two f",
                                     c=4, two=2, k=4, f=128) for t in ift8]
                wct8v = [t.rearrange("p (c two n) -> p c two n",
                                     c=4, two=2, n=512) for t in wct8]
                wenct8v = [t.rearrange("p (i two f) -> p i two f",
                                       i=4, two=2, f=128) for t in wenct8]

                # enc_projT [a, (b,j,q)] += wenc_b + wdec_b
                for c in range(NB * PPAD // 512):
                    for i in range(NA):
                        ps = pspre.tile([128, 512], F32, name="eps", tag="mm")
                        for kk in range(NKK):
                            nc.tensor.matmul(
                                ps, wenct8v[kk][:, i], ift8v[kk][:, c],
                                start=(kk == 0), stop=(kk == NKK - 1),
                                perf_mode=DR)
                        nc.vector.tensor_scalar_add(
                            encp[i][:, ts(c, 512)], ps, encb[i])

                # IFW[(b,j,q), d4] = IF @ Wc.T
                for c in range(4):
                    for bj in range(NBJ):
                        ps = pspre.tile([128, 512], F32, name="wps", tag="mm")
                        for kk in range(NKK):
                            nc.tensor.matmul(
                                ps, ift8w[kk][:, bj // 4, bj % 4],
                                wct8v[kk][:, c],
                                start=(kk == 0), stop=(kk == NKK - 1),
                                perf_mode=DR)
                        dst = ifw[bj][:, ts(c, 512)]
                        if (bj + c) % 2 == 0:
                            nc.scalar.copy(out=dst, in_=ps)
                        else:
                            nc.vector.tensor_copy(out=dst, in_=ps)

            # ---------------- recurrence + tail ----------------
            with tc.tile_pool(name="rec", bufs=1) as rec, \
                 tc.tile_pool(name="psl", bufs=1, space="PSUM") as psl:
                att = [rec.tile([128, APAD], BF16, name=f"att{i}",
                                tag=f"att{i}") for i in range(NA)]
                fct = [rec.tile([128, VOCAB], BF16, name=f"fct{k}",
                                tag=f"fct{k}") for k in range(ND)]
                # zero the per-half pad columns once (the last image of each
                # half has its j=1 e-matmul read 60 cols past the half)
                for i in range(NA):
                    for h in range(2):
                        nc.vector.memset(
                            att[i][:, h * HPAD + HPACK:(h + 1) * HPAD], 0.0)
                for k in range(ND):
                    nc.sync.dma_start(out=fct[k], in_=fct_d[k])

                # strided views for the one-instr-per-i bias add
                encv = [encp[i].rearrange("p (b q) -> p b q", b=NB, q=PPAD)
                        [:, :, 0:P] for i in range(NA)]
                attv = [[att[i][:, h * HPAD:h * HPAD + HPACK]
                         .rearrange("p (b q) -> p b q", b=NBH, q=P)
                         for h in range(2)] for i in range(NA)]

                for t in range(S):
                    hof = t * NB
                    hofn = hof + NB
                    # one aux PSUM bank: hp [0:32], etp [32:48], rs [48:56],
                    # sum [56:64] (start=True poisons a whole bank, so all
                    # starts here precede any re-accumulation of live bytes)
                    aux = psl.tile([128, 64], F32, name="aux", tag="aux",
                                   bufs=2)
                    hp_ps = aux[:, 0:NA * NB]
                    etp = aux[:, 32:32 + NJ * NB]
                    rs_ps = aux[:, 48:48 + NB]
                    sum_ps = aux[0:1, 56:56 + NB]
                    # hproj (i-outer so tsa/tanh of i=0 start early; also
                    # keeps each i-region's start->accum sequence contiguous)
                    for i in range(NA):
                        for k in range(ND):
                            nc.tensor.matmul(
                                hp_ps[:, ts(i, NB)], wdect[k][:, ts(i, 128)],
                                HT[k][:, hof:hofn], start=(k == 0),
                                stop=(k == ND - 1), skip_group_check=True)
                        nc.vector.tensor_copy(out=hp_s[:, ts(i, NB)],
                                              in_=hp_ps[:, ts(i, NB)])
                    # gates: per-r PSUM tiles (tile==bank; keeps ctx writes
                    # and cell reads on different tiles so they pipeline).
                    # One ept matmul per r with start=True initializes the
                    # bank; then W_hh (k-outer: pipelines with the previous
                    # step's cell) and ctx accumulate.
                    g_r = [psl.tile([128, 4 * NB], F32, name=f"g{r}",
                                    tag=f"g{r}", bufs=1) for r in range(ND)]
                    for r in range(ND):
                        ec = (t * 4 + r) * 4 * NB
                        nc.tensor.matmul(g_r[r], i128,
                                         ept[:, ec:ec + 4 * NB],
                                         start=True, stop=False,
                                         skip_group_check=True)
                    for k in range(ND):
                        for g in range(4):
                            for r in range(ND):
                                nc.tensor.matmul(
                                    g_r[r][:, ts(g, NB)],
                                    whht[k][:, ts(g * 4 + r, 128)],
                                    HT[k][:, hof:hofn],
                                    start=False, stop=False,
                                    skip_group_check=True)
                    # etp bank-init: one start=True matmul zeroes all 16 e
                    # columns so the e-matmuls can accumulate i-outer (each
                    # i issued right after its tanh) in any order.
                    nc.tensor.matmul(etp, i128, zeros16, start=True,
                                     stop=False, skip_group_check=True)

                    # Attention + softmax + ctx in TWO image-halves: while
                    # half-0's ctx matmuls run on the PE, half-1's add/tanh
                    # spine runs on vector/scalar — hides the serial spine.
                    def e_mm(h, i):
                        # e_T[q, (j,b)] = V . att — contribution of one i
                        for bh in range(NBH):
                            cb = h * HPAD + bh * P
                            for j in range(NJ):
                                col = h * 8 + j * NBH + bh
                                nc.tensor.matmul(
                                    etp[:, col:col + 1],
                                    att[i][:, cb + j * 128:
                                           cb + j * 128 + 128],
                                    vt[i], start=False,
                                    stop=(i == NA - 1),
                                    skip_group_check=True)

                    def spine(h, with_e):
                        b0 = h * NBH
                        for i in range(NA):
                            hv = hp_s[:, i * NB + b0:i * NB + b0 + NBH] \
                                .unsqueeze(2).broadcast_to([128, NBH, P])
                            nc.vector.tensor_tensor(
                                out=attv[i][h],
                                in0=encv[i][:, b0:b0 + NBH],
                                in1=hv, op=OP.add)
                            # att = tanh(att_pre), in place
                            nc.scalar.activation(
                                att[i][:, h * HPAD:h * HPAD + HPACK],
                                att[i][:, h * HPAD:h * HPAD + HPACK],
                                AF.Tanh)
                            if with_e:
                                e_mm(h, i)

                    def sm_sum(h):
                        # softmax over P (no max-subtraction: |e| <= ~11)
                        nc.scalar.activation(et_exp[:, ts(h, 8)],
                                             etp[:, ts(h, 8)], AF.Exp)
                        nc.tensor.matmul(sum_ps[:, ts(h, NBH)],
                                         ones_col[0:128],
                                         et_exp[:, h * 8:h * 8 + NBH],
                                         start=True, stop=False,
                                         skip_group_check=True)
                        nc.tensor.matmul(sum_ps[:, ts(h, NBH)],
                                         ones_col[0:P - 128],
                                         et_exp[0:P - 128,
                                                h * 8 + NBH:h * 8 + 2 * NBH],
                                         start=False, stop=True,
                                         skip_group_check=True)
                        nc.vector.reciprocal(rsum_sb[:, ts(h, NBH)],
                                             sum_ps[:, ts(h, NBH)])

                    def sm_wt(h):
                        b0 = h * NBH
                        nc.tensor.matmul(rs_ps[:, ts(h, NBH)], ones_row,
                                         rsum_sb[:, ts(h, NBH)],
                                         start=True, stop=True,
                                         skip_group_check=True)
                        for j in range(NJ):
                            nc.vector.tensor_mul(
                                wt_sb[j][:, b0:b0 + NBH],
                                et_exp[:, h * 8 + j * NBH:
                                       h * 8 + j * NBH + NBH],
                                rs_ps[:, ts(h, NBH)])

                    def ctx(h, rr):
                        b0 = h * NBH
                        for r in rr:
                            for g in range(4):
                                m = g * 4 + r
                                for b in range(b0, b0 + NBH):
                                    for j in range(NJ):
                                        nc.tensor.matmul(
                                            g_r[r][:, g * NB + b:
                                                   g * NB + b + 1],
                                            ifw[b * NJ + j][:, ts(m, 128)],
                                            wt_sb[j][:, b:b + 1],
                                            start=False, stop=(j == NJ - 1),
                                            skip_group_check=True)

                    spine(0, True)     # tt/tanh/e for images 0-3
                    sm_sum(0)          # exp + sum + recip
                    sm_wt(0)           # rs broadcast + weight muls
                    spine(1, False)    # h1 tt/tanh overlap ctx-h0 (vec/scal)
                    ctx(0, (0, 1))
                    for i in range(NA):
                        e_mm(1, i)     # tanh-h1 done by now: no PE stall
                    sm_sum(1)
                    ctx(0, (2, 3))
                    sm_wt(1)
                    for r in range(ND):
                        ctx(1, (r,))
                        # cell for this r (all-tanh: sigmoid(x)=.5(1+tanh(x/2));
                        # H holds 2h, C holds 2c)
                        cs = slice(r * NB, r * NB + NB)
                        nc.scalar.activation(tI[:, cs], g_r[r][:, ts(0, NB)],
                                             AF.Tanh, scale=0.5)
                        nc.scalar.activation(tF[:, cs], g_r[r][:, ts(1, NB)],
                                             AF.Tanh, scale=0.5)
                        nc.scalar.activation(tG[:, cs], g_r[r][:, ts(2, NB)],
                                             AF.Tanh)
                        nc.scalar.activation(tO[:, cs], g_r[r][:, ts(3, NB)],
                                             AF.Tanh, scale=0.5)
                        nc.vector.scalar_tensor_tensor(
                            tA[:, cs], tF[:, cs], 1.0, cC[:, cs],
                            OP.add, OP.mult)
                        nc.vector.scalar_tensor_tensor(
                            tB[:, cs], tI[:, cs], 1.0, tG[:, cs],
                            OP.add, OP.mult)
                        nc.vector.scalar_tensor_tensor(
                            cC[:, cs], tA[:, cs], 0.5, tB[:, cs],
                            OP.mult, OP.add)
                        nc.scalar.activation(tA[:, cs], cC[:, cs],
                                             AF.Tanh, scale=0.5)
                        nc.vector.scalar_tensor_tensor(
                            HT[r][:, hofn:hofn + NB], tO[:, cs], 1.0,
                            tA[:, cs], OP.add, OP.mult)

                # ---------------- tail: logits ----------------
                with tc.tile_pool(name="pst", bufs=2, space="PSUM") as pst:
                    for m0, msz in ((0, 128), (128, S * NB - 128)):
                        for c in range(NVC):
                            ps = pst.tile([128, VC], F32, name="lps", tag="l")
                            for k in range(ND):
                                nc.tensor.matmul(
                                    ps[:msz],
                                    HT[k][:, NB + m0:NB + m0 + msz],
                                    fct[k][:, ts(c, VC)],
                                    start=(k == 0), stop=(k == ND - 1))
                            lg = rec.tile([128, VC], F32, name="lg", tag="lg",
                                          bufs=4)
                            if c % 2 == 0:
                                nc.scalar.copy(out=lg[:msz], in_=ps[:msz])
                            else:
                                nc.vector.tensor_copy(out=lg[:msz],
                                                      in_=ps[:msz])
                            nc.sync.dma_start(
                                out=out_d[m0:m0 + msz, ts(c, VC)],
                                in_=lg[:msz])

    nc.compile()
    _CACHE["nc"] = nc
    return nc


def _prep_core_inputs(image_feat, embproj, h0, c0, wct8, wenct8, whht, wdect,
                      vt, i128, fct, encb, core):
    bs = slice(core * NB, (core + 1) * NB)
    ifp = np.zeros((NB, PPAD, ENC), np.float32)
    ifp[:, :P, :] = image_feat[bs]
    # IF.T as e-tiles [NE, 128, (b,j,q)], then fp8 pair layout
    iftT = np.ascontiguousarray(
        ifp.reshape(NB * PPAD, ENC).T).reshape(NE, 128, NB * PPAD)
    ift8 = np.ascontiguousarray(
        iftT.reshape(NKK, 2, 128, 4, 512).transpose(0, 2, 3, 1, 4)
        .reshape(NKK, 128, 2 * NB * PPAD)).astype(F8)
    ep = embproj[bs]                                   # [8, 20, 2048]
    ept = np.ascontiguousarray(
        ep.transpose(2, 1, 0)                          # [2048, 20, 8]
        .reshape(4, 4, 128, S, NB)                     # [gate, r, row, t, b]
        .transpose(2, 3, 1, 0, 4)                      # [row, t, r, gate, b]
        .reshape(128, S * 16 * NB)).astype(BF)
    h0t = np.ascontiguousarray(
        (2.0 * h0[bs]).T).reshape(ND, 128, NB).astype(BF)
    c0t = np.ascontiguousarray(
        (2.0 * c0[bs]).T.reshape(ND, 128, NB).transpose(1, 0, 2)
        .reshape(128, ND * NB)).astype(np.float32)
    return dict(ift8=ift8, wct8=wct8, wenct8=wenct8, whht=whht, wdect=wdect,
                vt=vt, ept=ept, i128=i128, fct=fct, h0t=h0t, c0t=c0t,
                encb=encb)


def kernel(image_feat, captions_ids, wenc_w, wenc_b, wdec_w, wdec_b,
           V_w, V_b, embed_w, h0_w, h0_b, c0_w, c0_b,
           W_ih, b_ih, W_hh, b_hh, fc_w, fc_b):
    image_feat = np.asarray(image_feat, np.float32)
    ids = np.asarray(captions_ids).astype(np.int64)

    # host-side glue (cheap, not on the device critical path)
    emb_seq = np.asarray(embed_w, np.float32)[ids]            # [B, S, EMB]
    We = np.asarray(W_ih, np.float32)[:, ENC:]                # [D4, EMB]
    Wc = np.asarray(W_ih, np.float32)[:, :ENC]                # [D4, ENC]
    embproj = emb_seq @ We.T + (np.asarray(b_ih) + np.asarray(b_hh))
    avg = image_feat.mean(axis=1)
    h0 = np.maximum(avg @ np.asarray(h0_w, np.float32).T + h0_b, 0.0)
    c0 = np.maximum(avg @ np.asarray(c0_w, np.float32).T + c0_b, 0.0)

    # fp8 pair layouts for the DoubleRow pre-loop matmuls
    wctT = np.ascontiguousarray(Wc.T).reshape(NE, 128, D4)
    wct8 = np.ascontiguousarray(
        wctT.reshape(NKK, 2, 128, 4, 512).transpose(0, 2, 3, 1, 4)
        .reshape(NKK, 128, 2 * D4)).astype(F8)
    wencT = np.ascontiguousarray(
        np.asarray(wenc_w, np.float32).T).reshape(NE, 128, ATT)
    wenct8 = np.ascontiguousarray(
        wencT.reshape(NKK, 2, 128, 4, 128).transpose(0, 2, 3, 1, 4)
        .reshape(NKK, 128, 2 * ATT)).astype(F8)
    whht = np.ascontiguousarray(
        0.5 * np.asarray(W_hh, np.float32).T).astype(BF).reshape(ND, 128, D4)
    wdect = np.ascontiguousarray(
        0.5 * np.asarray(wdec_w, np.float32).T).astype(BF).reshape(
            ND, 128, ATT)
    vtt = np.ascontiguousarray(
        np.asarray(V_w, np.float32)[0]).astype(BF).reshape(NA, 128, 1)
    i128 = np.eye(128, dtype=BF)
    fct = np.ascontiguousarray(
        0.5 * np.asarray(fc_w, np.float32).T).astype(BF).reshape(
            ND, 128, VOCAB)
    encb = (np.asarray(wenc_b, np.float32)
            + np.asarray(wdec_b, np.float32)).reshape(NA, 128, 1)

    nc = _build_nc()
    in_maps = [
        _prep_core_inputs(image_feat, embproj, h0, c0, wct8, wenct8, whht,
                          wdect, vtt, i128, fct, encb, c)
        for c in range(NCORES)
    ]
    res = run_bass_kernel_spmd(nc, in_maps, core_ids=list(range(NCORES)),
                               trace=TRACE)
    if TRACE:
        _CACHE["last_results"] = res

    preds = np.empty((B, S, VOCAB), np.float32)
    for c in range(NCORES):
        lg = res.results[c]["out"].reshape(S, NB, VOCAB)
        preds[c * NB:(c + 1) * NB] = lg.transpose(1, 0, 2)
    preds += np.asarray(fc_b, np.float32)
    return preds


if __name__ == "__main__":
    sys.path.insert(0, os.path.dirname(os.path.abspath(__file__)))
    import reference

    inputs = reference.setup_inputs()
    inputs = {k: np.asarray(v) for k, v in inputs.items()}
    expected = np.asarray(reference.reference(**inputs))
    actual = kernel(**inputs)
    err = np.abs(actual - expected)
    rel = np.linalg.norm(actual - expected) / np.linalg.norm(expected)
    print("max abs err:", err.max(), "rel:", rel)
